# revision 1
# baseline (speedup 1.0000x reference)
# Trainium2 Bass kernel for MixedChunkAttention.
#
# Sharding: 8 cores = 4 batches x 2-way tensor-parallel split of INNER
# (E=2048 -> 1024 per core). Each core processes one full batch (the
# cross-chunk kv cumsum stays core-local) and one half of the inner dim;
# the host sums the two partial outputs per batch and adds bout.
#
# Per-core dataflow (chunked over G=16 chunks of C=256 positions):
#   xT chunk [D, C] streamed to SBUF (host pre-transposes x)
#   xhT  = silu(Win^T @ xT + bin)                  [H, C]   (PE + ACT)
#   qqT/qkT/lqT/lkT = per-partition affines of xhT          (DVE)
#   lk_nat = transpose(lkT)                        [C, H]   (PE transpose)
#   attnT[m,n] = mask(relu^2(qkT_m^T @ qqT))       [C, C]   (PE + ACT + DVE)
#   v    = silu(x @ Wv + bv)   natural [C, E']              (PE + ACT)
#   gT   = silu(Wg^T @ xT + bg)          [E', C]            (PE + ACT)
#   vqlT[e,:] = S[:,e]^T @ lqT + sum_m v[m,e]^T @ attnT[m]  (PE, fused psum accum)
#   oT   = vqlT * gT                                        (DVE)
#   S   += lk_nat^T @ v   (kv state update, after vql read) (PE + DVE)
#   out[c,:] += oT_e^T @ Wout[e,:]  over e-tiles            (PE)
#
# All matmuls run in float32r (reduced-precision fp32, 4x the fp32 rate).

import numpy as np

B, S, D = 4, 4096, 1024
C, H, E = 256, 128, 2048
G = S // C            # 16 chunks
ELOC = E // 2         # per-core inner slice
T = D // 128          # 8 d-tiles
ET = ELOC // 128      # 8 e-tiles
NCORES = 8

_CACHE = {}
import os as _os
_PS512_BUFS = int(_os.environ.get("PS512_BUFS", "4"))
_PS256_BUFS = int(_os.environ.get("PS256_BUFS", "4"))


def _build_nc(n_chunks=G, reps=1, with_bv=True):
    import concourse.mybir as mybir
    import concourse.tile as tile
    from concourse import bacc
    from concourse.masks import make_identity

    F32, F32R = mybir.dt.float32, mybir.dt.float32r
    AF = mybir.ActivationFunctionType
    OP = mybir.AluOpType

    nc = bacc.Bacc()
    xT_d = nc.declare_dram_parameter("xT", [128, T, S], F32R, isOutput=False)
    wv_d = nc.declare_dram_parameter("wv", [128, T, ELOC], F32R, isOutput=False)
    wg_d = nc.declare_dram_parameter("wg", [128, T, ELOC], F32R, isOutput=False)
    win_d = nc.declare_dram_parameter("win", [128, T, H], F32R, isOutput=False)
    wout_d = nc.declare_dram_parameter("wout", [128, ET, D], F32R, isOutput=False)
    bv_d = nc.declare_dram_parameter("bv", [1, ELOC], F32R, isOutput=False)
    bgt_d = nc.declare_dram_parameter("bgt", [128, ET], F32, isOutput=False)
    aff_d = nc.declare_dram_parameter("aff", [128, 9], F32, isOutput=False)
    msk_d = nc.declare_dram_parameter("masks", [128, 512], F32, isOutput=False)
    one_d = nc.declare_dram_parameter("ones", [1, 128], F32R, isOutput=False)
    zs_d = nc.declare_dram_parameter("zeros", [128, ELOC], F32R, isOutput=False)
    out_d = nc.declare_dram_parameter("out", [S, D], F32, isOutput=True)

    with tile.TileContext(nc) as tc:
        with tc.tile_pool(name="wpool", bufs=1) as wpool, \
             tc.tile_pool(name="spool", bufs=1) as spool, \
             tc.tile_pool(name="xtp", bufs=2) as xtp, \
             tc.tile_pool(name="vp", bufs=1) as vp, \
             tc.tile_pool(name="gp", bufs=1) as gp, \
             tc.tile_pool(name="otp", bufs=1) as otp, \
             tc.tile_pool(name="osp", bufs=1) as osp, \
             tc.tile_pool(name="smallp", bufs=2) as smallp, \
             tc.tile_pool(name="ps512", bufs=_PS512_BUFS, space="PSUM") as ps512, \
             tc.tile_pool(name="ps256", bufs=_PS256_BUFS, space="PSUM") as ps256:

            # ---- persistent tiles ----
            wv_sb = wpool.tile([128, T, ELOC], F32R, name="wv_sb")
            wg_sb = wpool.tile([128, T, ELOC], F32R, name="wg_sb")
            win_sb = wpool.tile([128, T, H], F32R, name="win_sb")
            wout_sb = wpool.tile([128, ET, D], F32R, name="wout_sb")
            bv_sb = wpool.tile([1, ELOC], F32R, name="bv_sb")
            bgt_sb = wpool.tile([128, ET], F32, name="bgt_sb")
            aff_sb = wpool.tile([128, 9], F32, name="aff_sb")
            msk_sb = wpool.tile([128, 512], F32, name="msk_sb")
            one_sb = wpool.tile([1, 128], F32R, name="one_sb")
            ident = wpool.tile([128, 128], F32, name="ident")
            nc.sync.dma_start(out=wv_sb[:], in_=wv_d[:])
            nc.sync.dma_start(out=wg_sb[:], in_=wg_d[:])
            nc.sync.dma_start(out=win_sb[:], in_=win_d[:])
            nc.sync.dma_start(out=wout_sb[:], in_=wout_d[:])
            nc.sync.dma_start(out=bv_sb[:], in_=bv_d[:])
            nc.sync.dma_start(out=bgt_sb[:], in_=bgt_d[:])
            nc.sync.dma_start(out=aff_sb[:], in_=aff_d[:])
            nc.sync.dma_start(out=msk_sb[:], in_=msk_d[:])
            nc.sync.dma_start(out=one_sb[:], in_=one_d[:])
            make_identity(nc, ident)

            St = spool.tile([128, ELOC], F32R, name="St")

            import contextlib
            rep_ctx = tc.For_i(0, reps) if reps > 1 else contextlib.nullcontext()
            with rep_ctx:
                nc.sync.dma_start(out=St[:], in_=zs_d[:])
                _chunk_body(nc, tc, n_chunks, with_bv, locals())

    nc.finalize()
    return nc


def _chunk_body(nc, tc, n_chunks, with_bv, env):
    import concourse.mybir as mybir
    F32, F32R = mybir.dt.float32, mybir.dt.float32r
    AF = mybir.ActivationFunctionType
    OP = mybir.AluOpType
    (xT_d, out_d) = (env["xT_d"], env["out_d"])
    (wv_sb, wg_sb, win_sb, wout_sb, bv_sb, bgt_sb, aff_sb, msk_sb, one_sb,
     ident, St) = (env[k] for k in
                   ["wv_sb", "wg_sb", "win_sb", "wout_sb", "bv_sb", "bgt_sb",
                    "aff_sb", "msk_sb", "one_sb", "ident", "St"])
    (xtp, vp, gp, otp, osp, smallp, ps512, ps256) = (
        env[k] for k in ["xtp", "vp", "gp", "otp", "osp", "smallp",
                         "ps512", "ps256"])
    assert n_chunks % 2 == 0
    for gp_i in range(n_chunks // 2):
            # ---- load x^T chunk-pair, paired xh + gate projections ----
            xt = xtp.tile([128, T, 2 * C], F32R, name="xt", tag="xt")
            nc.sync.dma_start(out=xt[:],
                              in_=xT_d[:, :, gp_i * 2 * C:(gp_i + 1) * 2 * C])

            xh2 = smallp.tile([128, 2 * C], F32, name="xh2", tag="xh2", bufs=1)
            xh_ps = ps512.tile([128, 2 * C], F32, name="xh_ps", tag="ps512")
            for t in range(T):
                nc.tensor.matmul(xh_ps[:], win_sb[:, t, :], xt[:, t, :],
                                 start=(t == 0), stop=(t == T - 1))
            nc.scalar.activation(xh2[:], xh_ps[:], AF.Silu,
                                 bias=aff_sb[:, 8:9])

            gT2 = gp.tile([128, ET, 2 * C], F32, name="gT2", tag="gT2")
            for et in range(ET):
                g_ps = ps512.tile([128, 2 * C], F32, name="g_ps", tag="ps512")
                for t in range(T):
                    nc.tensor.matmul(g_ps[:],
                                     wg_sb[:, t, et * 128:(et + 1) * 128],
                                     xt[:, t, :],
                                     start=(t == 0), stop=(t == T - 1))
                nc.scalar.activation(gT2[:, et, :], g_ps[:], AF.Silu,
                                     bias=bgt_sb[:, et:et + 1])

            for gi in range(2):
                g = gp_i * 2 + gi
                co = gi * C                      # column offset into pair slabs
                xh = xh2[:, co:co + C]

                # ---- affines ----
                qqT = smallp.tile([128, C], F32R, name="qqT", tag="qqT")
                qkT = smallp.tile([128, C], F32R, name="qkT", tag="qkT")
                lqT = smallp.tile([128, C], F32R, name="lqT", tag="lqT")
                lkT = smallp.tile([128, C], F32, name="lkT", tag="lkT", bufs=1)
                nc.vector.tensor_scalar(out=qqT[:], in0=xh[:],
                                        scalar1=aff_sb[:, 0:1], scalar2=aff_sb[:, 1:2],
                                        op0=OP.mult, op1=OP.add)
                nc.vector.tensor_scalar(out=qkT[:], in0=xh[:],
                                        scalar1=aff_sb[:, 2:3], scalar2=aff_sb[:, 3:4],
                                        op0=OP.mult, op1=OP.add)
                nc.vector.tensor_scalar(out=lqT[:], in0=xh[:],
                                        scalar1=aff_sb[:, 4:5], scalar2=aff_sb[:, 5:6],
                                        op0=OP.mult, op1=OP.add)
                nc.vector.tensor_scalar(out=lkT[:], in0=xh[:],
                                        scalar1=aff_sb[:, 6:7], scalar2=aff_sb[:, 7:8],
                                        op0=OP.mult, op1=OP.add)

                # ---- lk natural via PE transpose ----
                lkn = smallp.tile([128, 2, H], F32R, name="lkn", tag="lkn")
                for ci in range(2):
                    tr_ps = ps256.tile([128, 128], F32, name="tr_ps", tag="ps256")
                    nc.tensor.transpose(tr_ps[:], lkT[:, ci * 128:(ci + 1) * 128],
                                        ident[:])
                    nc.vector.tensor_copy(lkn[:, ci, :], tr_ps[:])

                # ---- chunk attention attnT[m, n] ----
                attnT = smallp.tile([128, 2, C], F32R, name="attnT", tag="attnT")
                for mi in range(2):
                    at_ps = ps256.tile([128, C], F32, name="at_ps", tag="ps256")
                    nc.tensor.matmul(at_ps[:], qkT[:, mi * 128:(mi + 1) * 128],
                                     qqT[:], start=True, stop=True)
                    rt = smallp.tile([128, C], F32, name="rt", tag="rt")
                    nc.scalar.activation(rt[:], at_ps[:], AF.Relu, bias=0.0)
                    nc.vector.tensor_tensor(out=rt[:], in0=rt[:],
                                            in1=msk_sb[:, mi * C:(mi + 1) * C],
                                            op=OP.mult)
                    nc.vector.tensor_tensor(out=attnT[:, mi, :], in0=rt[:],
                                            in1=rt[:], op=OP.mult)

                # ---- v natural [C, ELOC] ----
                v_sb = vp.tile([128, 2, ELOC], F32R, name="v_sb", tag="v_sb")
                for ci in range(2):
                    for e2 in range(2):
                        v_ps = ps512.tile([128, 512], F32, name="v_ps", tag="ps512")
                        if with_bv:
                            nc.tensor.matmul(v_ps[:], one_sb[0:1, :],
                                             bv_sb[0:1, e2 * 512:(e2 + 1) * 512],
                                             start=True, stop=False)
                        for t in range(T):
                            nc.tensor.matmul(
                                v_ps[:], xt[:, t, co + ci * 128:co + (ci + 1) * 128],
                                wv_sb[:, t, e2 * 512:(e2 + 1) * 512],
                                start=(t == 0 and not with_bv),
                                stop=(t == T - 1))
                        nc.scalar.activation(
                            v_sb[:, ci, e2 * 512:(e2 + 1) * 512], v_ps[:], AF.Silu,
                            bias=0.0)

                # ---- v_lin + v_quad fused into one psum accum, then gate ----
                oT = otp.tile([128, ET, C], F32R, name="oT", tag="oT")
                for et in range(ET):
                    vql_ps = ps256.tile([128, C], F32, name="vql_ps", tag="ps256")
                    nc.tensor.matmul(vql_ps[:],
                                     St[:, et * 128:(et + 1) * 128], lqT[:],
                                     start=True, stop=False)
                    for mi in range(2):
                        nc.tensor.matmul(
                            vql_ps[:],
                            v_sb[:, mi, et * 128:(et + 1) * 128],
                            attnT[:, mi, :],
                            start=False, stop=(mi == 1))
                    nc.vector.tensor_tensor(out=oT[:, et, :],
                                            in0=gT2[:, et, co:co + C],
                                            in1=vql_ps[:], op=OP.mult)

                # ---- kv state update S += lk_nat^T @ v (after vql read S) ----
                for e2 in range(2):
                    kv_ps = ps512.tile([128, 512], F32, name="kv_ps", tag="ps512")
                    for ci in range(2):
                        nc.tensor.matmul(kv_ps[:], lkn[:, ci, :],
                                         v_sb[:, ci, e2 * 512:(e2 + 1) * 512],
                                         start=(ci == 0), stop=(ci == 1))
                    nc.vector.tensor_tensor(out=St[:, e2 * 512:(e2 + 1) * 512],
                                            in0=St[:, e2 * 512:(e2 + 1) * 512],
                                            in1=kv_ps[:], op=OP.add)

                # ---- output projection out[c, :] = sum_e oT_e^T @ Wout ----
                ostage = osp.tile([128, 2, D], F32, name="ostage", tag="ostage")
                for ci in range(2):
                    for d2 in range(2):
                        o_ps = ps512.tile([128, 512], F32, name="o_ps", tag="ps512")
                        for et in range(ET):
                            nc.tensor.matmul(
                                o_ps[:],
                                oT[:, et, ci * 128:(ci + 1) * 128],
                                wout_sb[:, et, d2 * 512:(d2 + 1) * 512],
                                start=(et == 0), stop=(et == ET - 1))
                        nc.vector.tensor_copy(
                            ostage[:, ci, d2 * 512:(d2 + 1) * 512], o_ps[:])
                    nc.sync.dma_start(
                        out=out_d[g * C + ci * 128: g * C + (ci + 1) * 128, :],
                        in_=ostage[:, ci, :])


def _get_nc(n_chunks=G, reps=1, with_bv=True):
    key = ("nc", n_chunks, reps, with_bv)
    if key not in _CACHE:
        _CACHE[key] = _build_nc(n_chunks, reps, with_bv)
    return _CACHE[key]


def _prep_inputs(x, Wv, bv, Wg, bg, Win, bin_, Wout, bout,
                 g_qq, b_qq, g_qk, b_qk, g_lq, b_lq, g_lk, b_lk):
    f = np.float32
    scale = f(E) ** f(0.5)
    tri = np.triu(np.ones((128, 128), f))          # keep p <= col
    masks = np.zeros((128, 512), f)
    masks[:, 0:128] = tri
    masks[:, 128:256] = 1.0
    masks[:, 256:384] = 0.0
    masks[:, 384:512] = tri
    aff = np.stack([
        g_qq / scale, b_qq / scale, g_qk, b_qk,
        g_lq, b_lq, g_lk, b_lk, bin_], axis=1).astype(f)       # [128, 9]
    ones = np.ones((1, 128), f)
    zeros = np.zeros((128, ELOC), f)

    def dtile(w, n):          # [D, n] -> [128, T, n]
        return np.ascontiguousarray(w.reshape(T, 128, n).transpose(1, 0, 2))

    in_maps = []
    for core in range(NCORES):
        b, h = core // 2, core % 2
        sl = slice(h * ELOC, (h + 1) * ELOC)
        xT = np.ascontiguousarray(
            x[b].T.reshape(T, 128, S).transpose(1, 0, 2))      # [128, T, S]
        wout_l = np.ascontiguousarray(
            Wout[sl, :].reshape(ET, 128, D).transpose(1, 0, 2))  # [128, ET, D]
        in_maps.append({
            "xT": xT.astype(f),
            "wv": dtile(Wv[:, sl], ELOC).astype(f),
            "wg": dtile(Wg[:, sl], ELOC).astype(f),
            "win": dtile(Win, H).astype(f),
            "wout": wout_l.astype(f),
            "bv": bv[sl].reshape(1, ELOC).astype(f),
            "bgt": np.ascontiguousarray(bg[sl].reshape(ET, 128).T).astype(f),
            "aff": aff,
            "masks": masks,
            "ones": ones,
            "zeros": zeros,
        })
    return in_maps


def _run(inputs, trace=False, reps=1, **trace_kw):
    import time
    from concourse.bass_utils import run_bass_kernel_spmd
    with_bv = bool(np.any(np.asarray(inputs["bv"])))
    nc = _get_nc(G, reps, with_bv)
    in_maps = _prep_inputs(**inputs)
    # The axon-tunneled devices occasionally fault transiently
    # (NRT_EXEC_UNIT_UNRECOVERABLE); the pool recovers on a fresh attempt.
    last_exc = None
    for attempt in range(4):
        try:
            res = run_bass_kernel_spmd(nc, in_maps,
                                       core_ids=list(range(NCORES)),
                                       trace=trace, **trace_kw)
            break
        except Exception as e:  # noqa: BLE001
            last_exc = e
            if "UNAVAILABLE" not in str(e) and "unrecoverable" not in str(e):
                raise
            time.sleep(10 * (attempt + 1))
    else:
        raise last_exc
    bout = np.asarray(inputs["bout"], np.float32)
    out = np.zeros((B, S, D), np.float32)
    for core in range(NCORES):
        out[core // 2] += res.results[core]["out"]
    out += bout[None, None, :]
    return out, res


def kernel(**inputs) -> np.ndarray:
    inputs = {k: np.asarray(v) for k, v in inputs.items()}
    out, _ = _run(inputs)
    return out



# revision 35
# speedup vs baseline: 14.3775x; 14.3775x over previous
# Trainium2 Bass kernel for MixedChunkAttention.
#
# Sharding: 8 cores = 4 batches x 2-way tensor-parallel split of INNER
# (E=2048 -> 1024 per core). Each core processes one full batch (the
# cross-chunk kv cumsum stays core-local) and one half of the inner dim;
# the host sums the two partial outputs per batch and adds bout.
#
# Precision scheme (rel_l2 ~= 1.05e-2 vs f32 reference, gate is 2e-2):
#   Big GEMMs (x@Wg, x@Win, o@Wout) run in fp8e4m3 with the DoubleRow
#   perf mode (2 K-tiles per instruction, 0.5 cycles/row) using a 3-slot
#   hi/lo expansion: W.x ~= Wh.xh + Wl.xh + Wh.xl, where (xh, xl) and
#   (Wh, Wl) are fp8 value+residual pairs. This keeps each GEMM's error
#   at ~0.1% while running at 4x the fp32r matmul rate (75% of the slots
#   of a plain 1-slot fp8 GEMM).
#   x@Wv runs 1-slot fp8 (errors wash out through attention averaging).
#   The o@Wout input o is split hi/lo on the fly (DVE+ACT); the o
#   residual is scaled x32 to clear the fp8 subnormal floor, compensated
#   by a Wout/32 weight copy.
#   attn QK^T and the v_lin state matmul stay fp32r. attn weights, v,
#   and lin_k are fp8 (DoubleRow for the attn@v and k^T v matmuls).
#   Static scales: x*8, W*256 (Wout*16), lq*4, lk*4, attn*16, o/16.
#
# Per-core dataflow (chunked over G=16 chunks of C=256 positions):
#   x^T hi/lo fp8 chunk-pair streamed to SBUF (host pre-quantizes)
#   xhT  = silu(Win^T @ xT)  3-slot fp8     [H, 2C]   (PE + ACT)
#   qqT/qkT/lqT f32r, lkT fp8 = per-partition affines  (DVE)
#   lk_nat = transpose(lkT)                 [C, H]    (PE f32r transpose)
#   attnT[m,n] = fp8(relu(qkT^T @ qqT)^2 * 16)        (PE f32r, ACT, DVE)
#   v    = fp8(silu(x @ Wv)) 1-slot fp8, natural [C, E']  (PE + ACT)
#   gT   = silu(Wg^T @ xT)   3-slot fp8     [E', 2C]  (PE + ACT)
#   vqlT = St^T @ lqT (f32r) + v^T @ attnT (fp8 DoubleRow, psum-fused)
#   t    = (gT * vqlT)/256 = o/16; o_hi = fp8(t), o_lo = fp8(32(t-hi))
#   St  += lk_nat^T @ v  (fp8 DoubleRow; St stores kv*4 in f32r)
#   out[c,:] = o_hi^T@WoutH + o_hi^T@WoutL + o_lo^T@WoutH/32 -> bf16

import numpy as np

B, S, D = 4, 4096, 1024
C, H, E = 256, 128, 2048
G = S // C            # 16 chunks
ELOC = E // 2         # per-core inner slice
T = D // 128          # 8 d-tiles
ET = ELOC // 128      # 8 e-tiles
NCORES = 8

SX = 8.0              # x stored scale
SW = 256.0            # Wv/Wg/Win stored scale
SWO = 16.0            # Wout stored scale (psum = out exactly)
SA = 16.0             # attn stored scale (sqrt folded into mask values)
SLK = 4.0             # lin_k stored scale
SLQ = 4.0             # lin_q stored scale

_CACHE = {}


def _build_nc(n_chunks=G, reps=1, with_bv=True):
    import concourse.mybir as mybir
    import concourse.tile as tile
    from concourse import bacc
    F32, F32R, BF16 = mybir.dt.float32, mybir.dt.float32r, mybir.dt.bfloat16
    FP8 = mybir.dt.float8e4
    AF = mybir.ActivationFunctionType
    OP = mybir.AluOpType
    DR = mybir.MatmulPerfMode.DoubleRow

    nc = bacc.Bacc()
    xh_d = nc.declare_dram_parameter("xh8", [128, T, S], FP8, isOutput=False)
    xl_d = nc.declare_dram_parameter("xl8", [128, T, S], FP8, isOutput=False)
    wvh_d = nc.declare_dram_parameter("wvh", [128, T, ELOC], FP8, isOutput=False)
    wgh_d = nc.declare_dram_parameter("wgh", [128, T, ELOC], FP8, isOutput=False)
    wgl_d = nc.declare_dram_parameter("wgl", [128, T, ELOC], FP8, isOutput=False)
    wih_d = nc.declare_dram_parameter("wih", [128, T, H], FP8, isOutput=False)
    wil_d = nc.declare_dram_parameter("wil", [128, T, H], FP8, isOutput=False)
    woh_d = nc.declare_dram_parameter("woh", [128, ET, D], FP8, isOutput=False)
    wol_d = nc.declare_dram_parameter("wol", [128, ET, D], FP8, isOutput=False)
    woc_d = nc.declare_dram_parameter("woc", [128, ET, D], FP8, isOutput=False)
    bv_d = nc.declare_dram_parameter("bv", [1, ELOC], F32R, isOutput=False)
    one_d = nc.declare_dram_parameter("ones", [1, 128], F32R, isOutput=False)
    bgt_d = nc.declare_dram_parameter("bgt", [128, ET], F32, isOutput=False)
    aff_d = nc.declare_dram_parameter("aff", [128, 9], F32, isOutput=False)
    msk_d = nc.declare_dram_parameter("masks", [128, 2, C], F32, isOutput=False)
    id_d = nc.declare_dram_parameter("ident", [128, 128], F32R, isOutput=False)
    zs_d = nc.declare_dram_parameter("zeros", [128, ELOC], F32R, isOutput=False)
    out_d = nc.declare_dram_parameter("out", [S, D], BF16, isOutput=True)

    with tile.TileContext(nc) as tc:
        with tc.tile_pool(name="wpool", bufs=1) as wpool, \
             tc.tile_pool(name="spool", bufs=1) as spool, \
             tc.tile_pool(name="xtp", bufs=2) as xtp, \
             tc.tile_pool(name="vp", bufs=2) as vp, \
             tc.tile_pool(name="gp", bufs=1) as gp, \
             tc.tile_pool(name="otp", bufs=2) as otp, \
             tc.tile_pool(name="osp", bufs=2) as osp, \
             tc.tile_pool(name="smallp", bufs=2) as smallp, \
             tc.tile_pool(name="ps512", bufs=4, space="PSUM") as ps512, \
             tc.tile_pool(name="pso", bufs=2, space="PSUM") as pso, \
             tc.tile_pool(name="ps256", bufs=1, space="PSUM") as ps256:

            # ---- persistent tiles ----
            wvh_sb = wpool.tile([128, T, ELOC], FP8, name="wvh_sb")
            wgh_sb = wpool.tile([128, T, ELOC], FP8, name="wgh_sb")
            wgl_sb = wpool.tile([128, T, ELOC], FP8, name="wgl_sb")
            wih_sb = wpool.tile([128, T, H], FP8, name="wih_sb")
            wil_sb = wpool.tile([128, T, H], FP8, name="wil_sb")
            woh_sb = wpool.tile([128, ET, D], FP8, name="woh_sb")
            wol_sb = wpool.tile([128, ET, D], FP8, name="wol_sb")
            woc_sb = wpool.tile([128, ET, D], FP8, name="woc_sb")
            bv_sb = wpool.tile([1, ELOC], F32R, name="bv_sb")
            one_sb = wpool.tile([1, 128], F32R, name="one_sb")
            bgt_sb = wpool.tile([128, ET], F32, name="bgt_sb")
            aff_sb = wpool.tile([128, 9], F32, name="aff_sb")
            msk_sb = wpool.tile([128, 2, C], F32, name="msk_sb")
            ident = wpool.tile([128, 128], F32R, name="ident")
            scr = wpool.tile([128, 1], F32, name="scr")
            for sb, d in ((wih_sb, wih_d), (wil_sb, wil_d), (wgh_sb, wgh_d)):
                nc.sync.dma_start(out=sb[:], in_=d[:])
            for sb, d in ((aff_sb, aff_d), (wgl_sb, wgl_d), (wvh_sb, wvh_d)):
                nc.scalar.dma_start(out=sb[:], in_=d[:])
            for sb, d in ((msk_sb, msk_d), (ident, id_d),
                          (bgt_sb, bgt_d), (bv_sb, bv_d), (one_sb, one_d),
                          (woh_sb, woh_d), (wol_sb, wol_d), (woc_sb, woc_d)):
                nc.gpsimd.dma_start(out=sb[:], in_=d[:])

            St = spool.tile([128, ELOC], F32R, name="St")

            import contextlib
            rep_ctx = tc.For_i(0, reps) if reps > 1 else contextlib.nullcontext()
            with rep_ctx:
                nc.sync.dma_start(out=St[:], in_=zs_d[:])
                _chunk_body(nc, tc, n_chunks, with_bv, locals())

    nc.finalize()
    return nc


def _chunk_body(nc, tc, n_chunks, with_bv, env):
    import concourse.mybir as mybir
    F32, F32R, BF16 = mybir.dt.float32, mybir.dt.float32r, mybir.dt.bfloat16
    FP8 = mybir.dt.float8e4
    AF = mybir.ActivationFunctionType
    OP = mybir.AluOpType
    DR = mybir.MatmulPerfMode.DoubleRow
    (xh_d, xl_d, out_d) = (env["xh_d"], env["xl_d"], env["out_d"])
    (wvh_sb, wgh_sb, wgl_sb, wih_sb, wil_sb, woh_sb, wol_sb, woc_sb,
     bv_sb, one_sb, bgt_sb, aff_sb, msk_sb, ident, scr, St) = (
        env[k] for k in
        ["wvh_sb", "wgh_sb", "wgl_sb", "wih_sb", "wil_sb", "woh_sb",
         "wol_sb", "woc_sb", "bv_sb", "one_sb", "bgt_sb", "aff_sb",
         "msk_sb", "ident", "scr", "St"])
    (xtp, vp, gp, otp, osp, smallp, ps512, pso, ps256) = (
        env[k] for k in ["xtp", "vp", "gp", "otp", "osp", "smallp",
                         "ps512", "pso", "ps256"])
    T2 = T // 2
    ZIN = 1.0 / (SX * SW)     # psum of 3-slot W.x holds z * SX*SW
    assert n_chunks % 2 == 0

    def emit_out(g, ohi, olo):
        # ---- output projection, 3-slot fp8 DoubleRow (chunk g) ----
        # psum holds out exactly (SWO=16 vs o/16); staged to bf16 on the
        # ACT engine (DVE is the busier of the two).
        ostage = osp.tile([128, 2, D], BF16, name="ostage", tag="ostage")
        for ci in range(2):
            cb = slice(ci * 128, (ci + 1) * 128)
            for d2 in range(2):
                ds = slice(d2 * 512, (d2 + 1) * 512)
                o_ps = pso.tile([128, 512], F32, name="o_ps", tag="pso")
                for ep in range(ET // 2):
                    es = slice(2 * ep, 2 * ep + 2)
                    nc.tensor.matmul(o_ps[:], ohi[:, es, cb],
                                     woh_sb[:, es, ds],
                                     start=(ep == 0), stop=False,
                                     perf_mode=DR)
                for ep in range(ET // 2):
                    es = slice(2 * ep, 2 * ep + 2)
                    nc.tensor.matmul(o_ps[:], ohi[:, es, cb],
                                     wol_sb[:, es, ds],
                                     start=False, stop=False,
                                     perf_mode=DR)
                for ep in range(ET // 2):
                    es = slice(2 * ep, 2 * ep + 2)
                    nc.tensor.matmul(o_ps[:], olo[:, es, cb],
                                     woc_sb[:, es, ds],
                                     start=False, stop=(ep == ET // 2 - 1),
                                     perf_mode=DR)
                nc.scalar.activation(ostage[:, ci, ds], o_ps[:], AF.Copy,
                                     bias=0.0)
            nc.sync.dma_start(
                out=out_d[g * C + ci * 128: g * C + (ci + 1) * 128, :],
                in_=ostage[:, ci, :])

    pending = None            # (g, ohi, olo) awaiting output projection
    for gp_i in range(n_chunks // 2):
            # ---- load x^T hi/lo chunk-pair ----
            cs = slice(gp_i * 2 * C, (gp_i + 1) * 2 * C)
            xht = xtp.tile([128, T, 2 * C], FP8, name="xht", tag="xht")
            xlt = xtp.tile([128, T, 2 * C], FP8, name="xlt", tag="xlt")
            nc.gpsimd.dma_start(out=xht[:], in_=xh_d[:, :, cs])
            nc.gpsimd.dma_start(out=xlt[:], in_=xl_d[:, :, cs])

            # ---- xh projection, 3-slot fp8 DoubleRow ----
            xh2 = smallp.tile([128, 2 * C], F32, name="xh2", tag="xh2", bufs=1)
            xh_ps = ps512.tile([128, 2 * C], F32, name="xh_ps", tag="ps512")
            for kp in range(T2):
                ks = slice(2 * kp, 2 * kp + 2)
                nc.tensor.matmul(xh_ps[:], wih_sb[:, ks, :], xht[:, ks, :],
                                 start=(kp == 0), stop=False, perf_mode=DR)
            for kp in range(T2):
                ks = slice(2 * kp, 2 * kp + 2)
                nc.tensor.matmul(xh_ps[:], wil_sb[:, ks, :], xht[:, ks, :],
                                 start=False, stop=False, perf_mode=DR)
            for kp in range(T2):
                ks = slice(2 * kp, 2 * kp + 2)
                nc.tensor.matmul(xh_ps[:], wih_sb[:, ks, :], xlt[:, ks, :],
                                 start=False, stop=(kp == T2 - 1), perf_mode=DR)
            nc.scalar.activation(xh2[:], xh_ps[:], AF.Silu,
                                 bias=aff_sb[:, 8:9], scale=ZIN)

            # ---- gate projection, 3-slot fp8 DoubleRow ----
            gT2 = gp.tile([128, ET, 2 * C], F32, name="gT2", tag="gT2")
            for et in range(ET):
                es = slice(et * 128, (et + 1) * 128)
                g_ps = ps512.tile([128, 2 * C], F32, name="g_ps", tag="ps512")
                for kp in range(T2):
                    ks = slice(2 * kp, 2 * kp + 2)
                    nc.tensor.matmul(g_ps[:], wgh_sb[:, ks, es], xht[:, ks, :],
                                     start=(kp == 0), stop=False, perf_mode=DR)
                for kp in range(T2):
                    ks = slice(2 * kp, 2 * kp + 2)
                    nc.tensor.matmul(g_ps[:], wgl_sb[:, ks, es], xht[:, ks, :],
                                     start=False, stop=False, perf_mode=DR)
                for kp in range(T2):
                    ks = slice(2 * kp, 2 * kp + 2)
                    nc.tensor.matmul(g_ps[:], wgh_sb[:, ks, es], xlt[:, ks, :],
                                     start=False, stop=(kp == T2 - 1),
                                     perf_mode=DR)
                nc.scalar.activation(gT2[:, et, :], g_ps[:], AF.Silu,
                                     bias=bgt_sb[:, et:et + 1], scale=ZIN)

            for gi in range(2):
                g = gp_i * 2 + gi
                co = gi * C                      # column offset into pair slabs
                xh = xh2[:, co:co + C]

                # ---- v natural [C, ELOC], 1-slot fp8 DoubleRow ----
                # (emitted first: independent of the xh2 affine chain)
                v_sb = vp.tile([128, 2, ELOC], FP8, name="v_sb", tag="v_sb")
                for ci in range(2):
                    cb = slice(co + ci * 128, co + (ci + 1) * 128)
                    for e2 in range(2):
                        v_ps = ps512.tile([128, 512], F32, name="v_ps",
                                          tag="ps512")
                        if with_bv:
                            nc.tensor.matmul(v_ps[:], one_sb[0:1, :],
                                             bv_sb[0:1, e2 * 512:(e2 + 1) * 512],
                                             start=True, stop=False)
                        for kp in range(T2):
                            ks = slice(2 * kp, 2 * kp + 2)
                            nc.tensor.matmul(
                                v_ps[:], xht[:, ks, cb],
                                wvh_sb[:, ks, e2 * 512:(e2 + 1) * 512],
                                start=(kp == 0 and not with_bv),
                                stop=(kp == T2 - 1), perf_mode=DR)
                        nc.scalar.activation(
                            v_sb[:, ci, e2 * 512:(e2 + 1) * 512], v_ps[:],
                            AF.Silu, bias=0.0, scale=ZIN)

                # ---- affines (lq*4 and lk*4 folded into aff) ----
                qqT = smallp.tile([128, C], F32R, name="qqT", tag="qqT")
                qkT = smallp.tile([128, C], F32R, name="qkT", tag="qkT")
                lqT = smallp.tile([128, C], F32R, name="lqT", tag="lqT")
                lkT = smallp.tile([128, C], F32R, name="lkT", tag="lkT", bufs=1)
                nc.vector.tensor_scalar(out=qqT[:], in0=xh[:],
                                        scalar1=aff_sb[:, 0:1], scalar2=aff_sb[:, 1:2],
                                        op0=OP.mult, op1=OP.add)
                nc.vector.tensor_scalar(out=qkT[:], in0=xh[:],
                                        scalar1=aff_sb[:, 2:3], scalar2=aff_sb[:, 3:4],
                                        op0=OP.mult, op1=OP.add)
                nc.vector.tensor_scalar(out=lqT[:], in0=xh[:],
                                        scalar1=aff_sb[:, 4:5], scalar2=aff_sb[:, 5:6],
                                        op0=OP.mult, op1=OP.add)
                nc.vector.tensor_scalar(out=lkT[:], in0=xh[:],
                                        scalar1=aff_sb[:, 6:7], scalar2=aff_sb[:, 7:8],
                                        op0=OP.mult, op1=OP.add)

                # ---- lk natural via PE transpose (f32r), cvt to fp8 ----
                lkn = smallp.tile([128, 2, H], FP8, name="lkn", tag="lkn")
                tr_ps = ps256.tile([128, 2, 128], F32R, name="tr_ps",
                                   tag="trps")
                for ci in range(2):
                    nc.tensor.matmul(tr_ps[:, ci, :],
                                     lkT[:, ci * 128:(ci + 1) * 128],
                                     ident[:], is_transpose=True,
                                     start=(ci == 0), stop=(ci == 1))
                nc.vector.tensor_copy(lkn[:, 0:2, :], tr_ps[:, 0:2, :])

                # ---- chunk attention attnT[m, n], stored attn*16 ----
                attnT = smallp.tile([128, 2, C], FP8, name="attnT", tag="attnT")
                at_ps = ps256.tile([128, 2, C], F32, name="at_ps", tag="atps")
                for mi in range(2):
                    nc.tensor.matmul(at_ps[:, mi, :],
                                     qkT[:, mi * 128:(mi + 1) * 128],
                                     qqT[:], start=(mi == 0), stop=(mi == 1))
                rt = smallp.tile([128, 2, C], F32, name="rt", tag="rt")
                nc.scalar.activation(rt[:], at_ps[:, 0:2, :], AF.Relu,
                                     bias=0.0)
                nc.vector.tensor_tensor(out=rt[:], in0=rt[:],
                                        in1=msk_sb[:, 0:2, :],
                                        op=OP.mult)
                nc.vector.tensor_tensor(out=attnT[:, 0:2, :], in0=rt[:],
                                        in1=rt[:], op=OP.mult)

                # ---- vql = lin (f32r) + quad (fp8 DR); o hi/lo split ----
                # All lin matmuls first (only need St + lqT) so the PE has
                # cover while the relu/mask/square chain produces attnT.
                # t/ohi/olo ops run et-pair wide; psum tiles are [128,2,C].
                ohi = otp.tile([128, ET, C], FP8, name="ohi", tag="ohi")
                olo = otp.tile([128, ET, C], FP8, name="olo", tag="olo")
                vql_pss = []
                for ep in range(ET // 2):
                    vql_ps = ps512.tile([128, 2, C], F32, name="vql_ps",
                                        tag="ps512")
                    vql_pss.append(vql_ps)
                    for j in range(2):
                        et = 2 * ep + j
                        es = slice(et * 128, (et + 1) * 128)
                        nc.tensor.matmul(vql_ps[:, j, :], St[:, es], lqT[:],
                                         start=(j == 0), stop=False)

                # ---- kv state update St += lk_nat^T @ v (fp8 DR) ----
                # Emitted before the quad matmuls so the St add lands ahead
                # of the tsb/olo chain in the DVE queue (next chunk's lin
                # matmuls wait on it).
                for e2 in range(2):
                    kv_ps = ps512.tile([128, 512], F32, name="kv_ps",
                                       tag="ps512")
                    nc.tensor.matmul(kv_ps[:], lkn[:, 0:2, :],
                                     v_sb[:, 0:2, e2 * 512:(e2 + 1) * 512],
                                     start=True, stop=True, perf_mode=DR)
                    nc.vector.tensor_tensor(out=St[:, e2 * 512:(e2 + 1) * 512],
                                            in0=St[:, e2 * 512:(e2 + 1) * 512],
                                            in1=kv_ps[:], op=OP.add)

                for ep in range(ET // 2):
                    vql_ps = vql_pss[ep]
                    for j in range(2):
                        et = 2 * ep + j
                        es = slice(et * 128, (et + 1) * 128)
                        nc.tensor.matmul(vql_ps[:, j, :], v_sb[:, 0:2, es],
                                         attnT[:, 0:2, :],
                                         start=False, stop=(j == 1),
                                         perf_mode=DR)
                    ee = slice(2 * ep, 2 * ep + 2)
                    # tsb = o*16; ohi = fp8(o/16); u = o*2 = 32*(o/16);
                    # olo = fp8(u - 32*ohi) = fp8(32*(o/16 - ohi)).
                    tsb = smallp.tile([128, 2, C], F32, name="tsb", tag="tsb")
                    usb = smallp.tile([128, 2, C], F32, name="usb", tag="usb")
                    nc.vector.tensor_tensor(
                        out=tsb[:], in0=gT2[:, ee, co:co + C],
                        in1=vql_ps[:, 0:2, :], op=OP.mult)
                    nc.scalar.activation(ohi[:, ee, :], tsb[:], AF.Copy,
                                         bias=0.0, scale=1.0 / 256.0)
                    nc.gpsimd.tensor_scalar_mul(usb[:], tsb[:], 1.0 / 8.0)
                    nc.vector.scalar_tensor_tensor(
                        out=olo[:, ee, :], in0=ohi[:, ee, :], scalar=-32.0,
                        in1=usb[:], op0=OP.mult, op1=OP.add)

                # ---- previous chunk's output projection (pipelined) ----
                if pending is not None:
                    emit_out(*pending)
                pending = (g, ohi, olo)

    if pending is not None:
        emit_out(*pending)


def _get_nc(n_chunks=G, reps=1, with_bv=True):
    key = ("nc", n_chunks, reps, with_bv)
    if key not in _CACHE:
        _CACHE[key] = _build_nc(n_chunks, reps, with_bv)
    return _CACHE[key]


def _fp8(a):
    import ml_dtypes
    return np.asarray(a, np.float32).astype(ml_dtypes.float8_e4m3)


def _prep_inputs(x, Wv, bv, Wg, bg, Win, bin_, Wout, bout,
                 g_qq, b_qq, g_qk, b_qk, g_lq, b_lq, g_lk, b_lk):
    f = np.float32
    scale = f(E) ** f(0.5)
    tri = np.triu(np.ones((128, 128), f))          # keep p <= col
    sa = f(SA) ** f(0.5)                           # folded into mask values
    masks = np.zeros((128, 512), f)
    masks[:, 0:128] = tri * sa
    masks[:, 128:256] = sa
    masks[:, 256:384] = 0.0
    masks[:, 384:512] = tri * sa
    aff = np.stack([
        g_qq / scale, b_qq / scale, g_qk, b_qk,
        g_lq * SLQ, b_lq * SLQ, g_lk * SLK, b_lk * SLK, bin_],
        axis=1).astype(f)                          # [128, 9]
    ones = np.ones((1, 128), f)
    zeros = np.zeros((128, ELOC), f)

    def dtile(w, n):          # [D, n] -> [128, T, n]
        return np.ascontiguousarray(w.reshape(T, 128, n).transpose(1, 0, 2))

    def wsplit(w, s):
        hi = _fp8(w * s)
        lo = _fp8(w * s - hi.astype(np.float32))
        return hi, lo

    x = np.asarray(x, f)
    WgH, WgL = wsplit(np.asarray(Wg, f), SW)
    WiH, WiL = wsplit(np.asarray(Win, f), SW)
    WvH = _fp8(np.asarray(Wv, f) * SW)
    WoH, WoL = wsplit(np.asarray(Wout, f), SWO)
    WoC = _fp8(np.asarray(Wout, f) * (SWO / 32.0))

    in_maps = []
    for core in range(NCORES):
        b, h = core // 2, core % 2
        sl = slice(h * ELOC, (h + 1) * ELOC)
        xb = x[b] * SX                                 # [S, D] scaled
        xh8 = _fp8(xb)
        xl8 = _fp8(xb - xh8.astype(f))
        def xtile(a):          # [S, D] fp8 -> [128, T, S]
            return np.ascontiguousarray(
                a.T.reshape(T, 128, S).transpose(1, 0, 2))
        def otile(w):          # [ELOC, D] fp8 -> [128, ET, D]
            return np.ascontiguousarray(
                w[sl, :].reshape(ET, 128, D).transpose(1, 0, 2))
        in_maps.append({
            "xh8": xtile(xh8),
            "xl8": xtile(xl8),
            "wvh": dtile(WvH[:, sl], ELOC),
            "wgh": dtile(WgH[:, sl], ELOC),
            "wgl": dtile(WgL[:, sl], ELOC),
            "wih": dtile(WiH, H),
            "wil": dtile(WiL, H),
            "woh": otile(WoH),
            "wol": otile(WoL),
            "woc": otile(WoC),
            "bv": (np.asarray(bv, f) * (SX * SW))[sl].reshape(1, ELOC),
            "ones": ones,
            "bgt": np.ascontiguousarray(
                np.asarray(bg, f)[sl].reshape(ET, 128).T),
            "aff": aff,
            "masks": masks.reshape(128, 2, C),
            "ident": np.eye(128, dtype=f),
            "zeros": zeros,
        })
    return in_maps


def _run(inputs, trace=False, reps=1, **trace_kw):
    import time
    from concourse.bass_utils import run_bass_kernel_spmd
    with_bv = bool(np.any(np.asarray(inputs["bv"])))
    nc = _get_nc(G, reps, with_bv)
    in_maps = _prep_inputs(**inputs)
    # The axon-tunneled devices occasionally fault transiently
    # (NRT_EXEC_UNIT_UNRECOVERABLE); the pool recovers on a fresh attempt.
    last_exc = None
    for attempt in range(4):
        try:
            res = run_bass_kernel_spmd(nc, in_maps,
                                       core_ids=list(range(NCORES)),
                                       trace=trace, **trace_kw)
            break
        except Exception as e:  # noqa: BLE001
            last_exc = e
            if "UNAVAILABLE" not in str(e) and "unrecoverable" not in str(e):
                raise
            time.sleep(10 * (attempt + 1))
    else:
        raise last_exc
    bout = np.asarray(inputs["bout"], np.float32)
    out = np.zeros((B, S, D), np.float32)
    for core in range(NCORES):
        out[core // 2] += res.results[core]["out"].astype(np.float32)
    out += bout[None, None, :]
    return out, res


def kernel(**inputs) -> np.ndarray:
    inputs = {k: np.asarray(v) for k, v in inputs.items()}
    out, _ = _run(inputs)
    return out


# revision 36
# speedup vs baseline: 16.9568x; 1.1794x over previous
# Trainium2 Bass kernel for MixedChunkAttention.
#
# Sharding: 8 cores = 4 batches x 2-way tensor-parallel split of INNER
# (E=2048 -> 1024 per core). Each core processes one full batch (the
# cross-chunk kv cumsum stays core-local) and one half of the inner dim;
# the host sums the two partial outputs per batch and adds bout.
#
# Precision scheme (rel_l2 ~= 1e-2 vs f32 reference, gate is 2e-2):
#   x@Wv runs 1-slot fp8e4m3 with the DoubleRow perf mode (2 K-tiles per
#   instruction -> 2x the fp32r matmul rate; measured on hw, the cost
#   model's 4x is wrong). Its quantization error washes out through the
#   attention averaging. The attn@v (quad) and k^T v (kv) matmuls are
#   also fp8 DoubleRow (attn weights, v, lin_k stored fp8).
#   Everything precision-critical (x@Wg, x@Win, attn QK^T, the v_lin
#   state matmul, o@Wout) stays fp32r.
#   Static scales: x_hi*8, Wv*256, lq*4, lk*4, attn*16, Wout/16 (so the
#   out psum holds the final value exactly); o stored as o*16 in f32r.
#
# Per-core dataflow (chunked over G=16 chunks of C=256 positions):
#   xT f32r + x_hi fp8 chunk-pair streamed to SBUF (host pre-quantizes)
#   xhT  = silu(Win^T @ xT)  f32r            [H, 2C]   (PE + ACT)
#   qqT/qkT/lqT/lkT f32r = per-partition affines       (DVE)
#   lk_nat = fp8(transpose(lkT))            [C, H]    (PE f32r transpose)
#   attnT[m,n] = fp8(relu(qkT^T @ qqT)^2 * 16)        (PE f32r, ACT, DVE)
#   v    = fp8(silu(x_hi @ Wv8))  natural [C, E']     (PE fp8 DR + ACT)
#   gT   = silu(Wg^T @ xT)   f32r            [E', 2C]  (PE + ACT)
#   vqlT = St^T @ lqT (f32r) + v^T @ attnT (fp8 DR, psum-fused)
#   oT   = (gT * vqlT) f32r (= o*16)                  (DVE)
#   St  += lk_nat^T @ v  (fp8 DR; St stores kv*4 in f32r)
#   out[c,:] = oT^T @ (Wout/16)  f32r -> bf16 stage -> DMA
#
# The output projection of chunk g is emitted during chunk g+1 so the
# in-order PE queue always has independent work while the oT chain
# (DVE/ACT) drains; kv is emitted before the quad matmuls so the St
# update lands ahead of the oT ops in the DVE queue.

import numpy as np

B, S, D = 4, 4096, 1024
C, H, E = 256, 128, 2048
G = S // C            # 16 chunks
ELOC = E // 2         # per-core inner slice
T = D // 128          # 8 d-tiles
ET = ELOC // 128      # 8 e-tiles
NCORES = 8

SX = 8.0              # x_hi stored scale
SW = 256.0            # Wv stored scale
SA = 16.0             # attn stored scale (sqrt folded into mask values)
SLK = 4.0             # lin_k stored scale
SLQ = 4.0             # lin_q stored scale
# oT holds o*16 (= vql*16 * gate); Wout shipped as Wout/16.

_CACHE = {}


def _build_nc(n_chunks=G, reps=1, with_bv=True):
    import concourse.mybir as mybir
    import concourse.tile as tile
    from concourse import bacc

    F32, F32R, BF16 = mybir.dt.float32, mybir.dt.float32r, mybir.dt.bfloat16
    FP8 = mybir.dt.float8e4
    AF = mybir.ActivationFunctionType
    OP = mybir.AluOpType
    DR = mybir.MatmulPerfMode.DoubleRow

    nc = bacc.Bacc()
    xt_d = nc.declare_dram_parameter("xT", [128, T, S], F32R, isOutput=False)
    xh_d = nc.declare_dram_parameter("xh8", [128, T, S], FP8, isOutput=False)
    wvh_d = nc.declare_dram_parameter("wvh", [128, T, ELOC], FP8, isOutput=False)
    wg_d = nc.declare_dram_parameter("wg", [128, T, ELOC], F32R, isOutput=False)
    win_d = nc.declare_dram_parameter("win", [128, T, H], F32R, isOutput=False)
    wout_d = nc.declare_dram_parameter("wout", [128, ET, D], F32R,
                                       isOutput=False)
    bv_d = nc.declare_dram_parameter("bv", [1, ELOC], F32R, isOutput=False)
    one_d = nc.declare_dram_parameter("ones", [1, 128], F32R, isOutput=False)
    bgt_d = nc.declare_dram_parameter("bgt", [128, ET], F32, isOutput=False)
    aff_d = nc.declare_dram_parameter("aff", [128, 9], F32, isOutput=False)
    msk_d = nc.declare_dram_parameter("masks", [128, 2, C], F32, isOutput=False)
    id_d = nc.declare_dram_parameter("ident", [128, 128], F32R, isOutput=False)
    zs_d = nc.declare_dram_parameter("zeros", [128, ELOC], F32R, isOutput=False)
    out_d = nc.declare_dram_parameter("out", [S, D], BF16, isOutput=True)

    with tile.TileContext(nc) as tc:
        with tc.tile_pool(name="wpool", bufs=1) as wpool, \
             tc.tile_pool(name="spool", bufs=1) as spool, \
             tc.tile_pool(name="xtp", bufs=2) as xtp, \
             tc.tile_pool(name="vp", bufs=2) as vp, \
             tc.tile_pool(name="gp", bufs=1) as gp, \
             tc.tile_pool(name="otp", bufs=2) as otp, \
             tc.tile_pool(name="osp", bufs=2) as osp, \
             tc.tile_pool(name="smallp", bufs=2) as smallp, \
             tc.tile_pool(name="ps512", bufs=4, space="PSUM") as ps512, \
             tc.tile_pool(name="pso", bufs=2, space="PSUM") as pso, \
             tc.tile_pool(name="ps256", bufs=1, space="PSUM") as ps256:

            # ---- persistent tiles ----
            wvh_sb = wpool.tile([128, T, ELOC], FP8, name="wvh_sb")
            wg_sb = wpool.tile([128, T, ELOC], F32R, name="wg_sb")
            win_sb = wpool.tile([128, T, H], F32R, name="win_sb")
            wout_sb = wpool.tile([128, ET, D], F32R, name="wout_sb")
            bv_sb = wpool.tile([1, ELOC], F32R, name="bv_sb")
            one_sb = wpool.tile([1, 128], F32R, name="one_sb")
            bgt_sb = wpool.tile([128, ET], F32, name="bgt_sb")
            aff_sb = wpool.tile([128, 9], F32, name="aff_sb")
            msk_sb = wpool.tile([128, 2, C], F32, name="msk_sb")
            ident = wpool.tile([128, 128], F32R, name="ident")
            for sb, d in ((win_sb, win_d), (wg_sb, wg_d)):
                nc.sync.dma_start(out=sb[:], in_=d[:])
            for sb, d in ((aff_sb, aff_d), (wvh_sb, wvh_d), (wout_sb, wout_d)):
                nc.scalar.dma_start(out=sb[:], in_=d[:])
            for sb, d in ((msk_sb, msk_d), (ident, id_d), (bgt_sb, bgt_d),
                          (bv_sb, bv_d), (one_sb, one_d)):
                nc.gpsimd.dma_start(out=sb[:], in_=d[:])

            St = spool.tile([128, ELOC], F32R, name="St")

            import contextlib
            rep_ctx = tc.For_i(0, reps) if reps > 1 else contextlib.nullcontext()
            with rep_ctx:
                nc.sync.dma_start(out=St[:], in_=zs_d[:])
                _chunk_body(nc, tc, n_chunks, with_bv, locals())

    nc.finalize()
    return nc


def _chunk_body(nc, tc, n_chunks, with_bv, env):
    import concourse.mybir as mybir
    F32, F32R, BF16 = mybir.dt.float32, mybir.dt.float32r, mybir.dt.bfloat16
    FP8 = mybir.dt.float8e4
    AF = mybir.ActivationFunctionType
    OP = mybir.AluOpType
    DR = mybir.MatmulPerfMode.DoubleRow
    (xt_d, xh_d, out_d) = (env["xt_d"], env["xh_d"], env["out_d"])
    (wvh_sb, wg_sb, win_sb, wout_sb, bv_sb, one_sb, bgt_sb, aff_sb,
     msk_sb, ident, St) = (
        env[k] for k in
        ["wvh_sb", "wg_sb", "win_sb", "wout_sb", "bv_sb", "one_sb",
         "bgt_sb", "aff_sb", "msk_sb", "ident", "St"])
    (xtp, vp, gp, otp, osp, smallp, ps512, pso, ps256) = (
        env[k] for k in ["xtp", "vp", "gp", "otp", "osp", "smallp",
                         "ps512", "pso", "ps256"])
    T2 = T // 2
    ZIN = 1.0 / (SX * SW)     # v psum holds z * SX*SW
    assert n_chunks % 2 == 0

    def emit_out(g, oT):
        # ---- output projection f32r (chunk g); psum = out exactly ----
        ostage = osp.tile([128, 2, D], BF16, name="ostage", tag="ostage")
        for ci in range(2):
            cb = slice(ci * 128, (ci + 1) * 128)
            for d2 in range(2):
                ds = slice(d2 * 512, (d2 + 1) * 512)
                o_ps = pso.tile([128, 512], F32, name="o_ps", tag="pso")
                for et in range(ET):
                    nc.tensor.matmul(o_ps[:], oT[:, et, cb],
                                     wout_sb[:, et, ds],
                                     start=(et == 0), stop=(et == ET - 1))
                nc.scalar.activation(ostage[:, ci, ds], o_ps[:], AF.Copy,
                                     bias=0.0)
            nc.sync.dma_start(
                out=out_d[g * C + ci * 128: g * C + (ci + 1) * 128, :],
                in_=ostage[:, ci, :])

    pending = None            # (g, oT) awaiting output projection
    for gp_i in range(n_chunks // 2):
            # ---- load x chunk-pair: f32r transposed + fp8 hi ----
            cs = slice(gp_i * 2 * C, (gp_i + 1) * 2 * C)
            xt = xtp.tile([128, T, 2 * C], F32R, name="xt", tag="xt")
            xht = xtp.tile([128, T, 2 * C], FP8, name="xht", tag="xht")
            nc.gpsimd.dma_start(out=xt[:], in_=xt_d[:, :, cs])
            nc.gpsimd.dma_start(out=xht[:], in_=xh_d[:, :, cs])

            # ---- xh projection f32r ----
            xh2 = smallp.tile([128, 2 * C], F32, name="xh2", tag="xh2", bufs=1)
            xh_ps = ps512.tile([128, 2 * C], F32, name="xh_ps", tag="ps512")
            for t in range(T):
                nc.tensor.matmul(xh_ps[:], win_sb[:, t, :], xt[:, t, :],
                                 start=(t == 0), stop=(t == T - 1))
            nc.scalar.activation(xh2[:], xh_ps[:], AF.Silu,
                                 bias=aff_sb[:, 8:9])

            # ---- gate projection f32r ----
            gT2 = gp.tile([128, ET, 2 * C], F32, name="gT2", tag="gT2")
            for et in range(ET):
                es = slice(et * 128, (et + 1) * 128)
                g_ps = ps512.tile([128, 2 * C], F32, name="g_ps", tag="ps512")
                for t in range(T):
                    nc.tensor.matmul(g_ps[:], wg_sb[:, t, es], xt[:, t, :],
                                     start=(t == 0), stop=(t == T - 1))
                nc.scalar.activation(gT2[:, et, :], g_ps[:], AF.Silu,
                                     bias=bgt_sb[:, et:et + 1])

            for gi in range(2):
                g = gp_i * 2 + gi
                co = gi * C                      # column offset into pair slabs
                xh = xh2[:, co:co + C]

                # ---- v natural [C, ELOC], 1-slot fp8 DoubleRow ----
                v_sb = vp.tile([128, 2, ELOC], FP8, name="v_sb", tag="v_sb")
                for ci in range(2):
                    cb = slice(co + ci * 128, co + (ci + 1) * 128)
                    for e2 in range(2):
                        v_ps = ps512.tile([128, 512], F32, name="v_ps",
                                          tag="ps512")
                        if with_bv:
                            nc.tensor.matmul(v_ps[:], one_sb[0:1, :],
                                             bv_sb[0:1, e2 * 512:(e2 + 1) * 512],
                                             start=True, stop=False)
                        for kp in range(T2):
                            ks = slice(2 * kp, 2 * kp + 2)
                            nc.tensor.matmul(
                                v_ps[:], xht[:, ks, cb],
                                wvh_sb[:, ks, e2 * 512:(e2 + 1) * 512],
                                start=(kp == 0 and not with_bv),
                                stop=(kp == T2 - 1), perf_mode=DR)
                        nc.scalar.activation(
                            v_sb[:, ci, e2 * 512:(e2 + 1) * 512], v_ps[:],
                            AF.Silu, bias=0.0, scale=ZIN)

                # ---- affines (lq*4 and lk*4 folded into aff) ----
                qqT = smallp.tile([128, C], F32R, name="qqT", tag="qqT")
                qkT = smallp.tile([128, C], F32R, name="qkT", tag="qkT")
                lqT = smallp.tile([128, C], F32R, name="lqT", tag="lqT")
                lkT = smallp.tile([128, C], F32R, name="lkT", tag="lkT", bufs=1)
                nc.vector.tensor_scalar(out=qqT[:], in0=xh[:],
                                        scalar1=aff_sb[:, 0:1], scalar2=aff_sb[:, 1:2],
                                        op0=OP.mult, op1=OP.add)
                nc.vector.tensor_scalar(out=qkT[:], in0=xh[:],
                                        scalar1=aff_sb[:, 2:3], scalar2=aff_sb[:, 3:4],
                                        op0=OP.mult, op1=OP.add)
                nc.vector.tensor_scalar(out=lqT[:], in0=xh[:],
                                        scalar1=aff_sb[:, 4:5], scalar2=aff_sb[:, 5:6],
                                        op0=OP.mult, op1=OP.add)
                nc.vector.tensor_scalar(out=lkT[:], in0=xh[:],
                                        scalar1=aff_sb[:, 6:7], scalar2=aff_sb[:, 7:8],
                                        op0=OP.mult, op1=OP.add)

                # ---- lk natural via PE transpose (f32r), cvt to fp8 ----
                lkn = smallp.tile([128, 2, H], FP8, name="lkn", tag="lkn")
                tr_ps = ps256.tile([128, 2, 128], F32R, name="tr_ps",
                                   tag="trps")
                for ci in range(2):
                    nc.tensor.matmul(tr_ps[:, ci, :],
                                     lkT[:, ci * 128:(ci + 1) * 128],
                                     ident[:], is_transpose=True,
                                     start=(ci == 0), stop=(ci == 1))
                nc.vector.tensor_copy(lkn[:, 0:2, :], tr_ps[:, 0:2, :])

                # ---- chunk attention attnT[m, n], stored attn*16 ----
                attnT = smallp.tile([128, 2, C], FP8, name="attnT", tag="attnT")
                at_ps = ps256.tile([128, 2, C], F32, name="at_ps", tag="atps")
                for mi in range(2):
                    nc.tensor.matmul(at_ps[:, mi, :],
                                     qkT[:, mi * 128:(mi + 1) * 128],
                                     qqT[:], start=(mi == 0), stop=(mi == 1))
                rt = smallp.tile([128, 2, C], F32, name="rt", tag="rt")
                nc.scalar.activation(rt[:], at_ps[:, 0:2, :], AF.Relu,
                                     bias=0.0)
                nc.vector.tensor_tensor(out=rt[:], in0=rt[:],
                                        in1=msk_sb[:, 0:2, :], op=OP.mult)
                nc.vector.tensor_tensor(out=attnT[:, 0:2, :], in0=rt[:],
                                        in1=rt[:], op=OP.mult)

                # ---- vql = lin (f32r) + quad (fp8 DR); oT = gate * vql ----
                # All lin matmuls first so the PE has cover while the
                # relu/mask/square chain produces attnT.
                oT = otp.tile([128, ET, C], F32R, name="oT", tag="oT")
                vql_pss = []
                for ep in range(ET // 2):
                    vql_ps = ps512.tile([128, 2, C], F32, name="vql_ps",
                                        tag="ps512")
                    vql_pss.append(vql_ps)
                    for j in range(2):
                        et = 2 * ep + j
                        es = slice(et * 128, (et + 1) * 128)
                        nc.tensor.matmul(vql_ps[:, j, :], St[:, es], lqT[:],
                                         start=(j == 0), stop=False)

                # ---- kv state update St += lk_nat^T @ v (fp8 DR) ----
                # Before the quad matmuls so the St add lands ahead of the
                # oT ops in the DVE queue (next chunk's lin matmuls wait).
                for e2 in range(2):
                    kv_ps = ps512.tile([128, 512], F32, name="kv_ps",
                                       tag="ps512")
                    nc.tensor.matmul(kv_ps[:], lkn[:, 0:2, :],
                                     v_sb[:, 0:2, e2 * 512:(e2 + 1) * 512],
                                     start=True, stop=True, perf_mode=DR)
                    nc.vector.tensor_tensor(out=St[:, e2 * 512:(e2 + 1) * 512],
                                            in0=St[:, e2 * 512:(e2 + 1) * 512],
                                            in1=kv_ps[:], op=OP.add)

                for ep in range(ET // 2):
                    vql_ps = vql_pss[ep]
                    for j in range(2):
                        et = 2 * ep + j
                        es = slice(et * 128, (et + 1) * 128)
                        nc.tensor.matmul(vql_ps[:, j, :], v_sb[:, 0:2, es],
                                         attnT[:, 0:2, :],
                                         start=False, stop=(j == 1),
                                         perf_mode=DR)
                    ee = slice(2 * ep, 2 * ep + 2)
                    nc.vector.tensor_tensor(
                        out=oT[:, ee, :], in0=gT2[:, ee, co:co + C],
                        in1=vql_ps[:, 0:2, :], op=OP.mult)

                # ---- previous chunk's output projection (pipelined) ----
                if pending is not None:
                    emit_out(*pending)
                pending = (g, oT)

    if pending is not None:
        emit_out(*pending)


def _get_nc(n_chunks=G, reps=1, with_bv=True):
    key = ("nc", n_chunks, reps, with_bv)
    if key not in _CACHE:
        _CACHE[key] = _build_nc(n_chunks, reps, with_bv)
    return _CACHE[key]


def _fp8(a):
    import ml_dtypes
    return np.asarray(a, np.float32).astype(ml_dtypes.float8_e4m3)


def _prep_inputs(x, Wv, bv, Wg, bg, Win, bin_, Wout, bout,
                 g_qq, b_qq, g_qk, b_qk, g_lq, b_lq, g_lk, b_lk):
    f = np.float32
    scale = f(E) ** f(0.5)
    tri = np.triu(np.ones((128, 128), f))          # keep p <= col
    sa = f(SA) ** f(0.5)                           # folded into mask values
    masks = np.zeros((128, 512), f)
    masks[:, 0:128] = tri * sa
    masks[:, 128:256] = sa
    masks[:, 256:384] = 0.0
    masks[:, 384:512] = tri * sa
    aff = np.stack([
        g_qq / scale, b_qq / scale, g_qk, b_qk,
        g_lq * SLQ, b_lq * SLQ, g_lk * SLK, b_lk * SLK, bin_],
        axis=1).astype(f)                          # [128, 9]
    ones = np.ones((1, 128), f)
    zeros = np.zeros((128, ELOC), f)

    def dtile(w, n):          # [D, n] -> [128, T, n]
        return np.ascontiguousarray(w.reshape(T, 128, n).transpose(1, 0, 2))

    x = np.asarray(x, f)
    WvH = _fp8(np.asarray(Wv, f) * SW)
    Wg = np.asarray(Wg, f)
    Win = np.asarray(Win, f)
    # oT holds o*16, so ship Wout/16 and the out psum is exact.
    Wout16 = np.asarray(Wout, f) / 16.0

    in_maps = []
    for core in range(NCORES):
        b, h = core // 2, core % 2
        sl = slice(h * ELOC, (h + 1) * ELOC)
        xb = x[b]                                      # [S, D]
        xh8 = _fp8(xb * SX)
        def xtile(a):          # [S, D] -> [128, T, S]
            return np.ascontiguousarray(
                a.T.reshape(T, 128, S).transpose(1, 0, 2))
        wout_l = np.ascontiguousarray(
            Wout16[sl, :].reshape(ET, 128, D).transpose(1, 0, 2))
        in_maps.append({
            "xT": xtile(xb).astype(f),
            "xh8": xtile(xh8),
            "wvh": dtile(WvH[:, sl], ELOC),
            "wg": dtile(Wg[:, sl], ELOC).astype(f),
            "win": dtile(Win, H).astype(f),
            "wout": wout_l.astype(f),
            "bv": (np.asarray(bv, f) * (SX * SW))[sl].reshape(1, ELOC),
            "ones": ones,
            "bgt": np.ascontiguousarray(
                np.asarray(bg, f)[sl].reshape(ET, 128).T),
            "aff": aff,
            "masks": masks.reshape(128, 2, C),
            "ident": np.eye(128, dtype=f),
            "zeros": zeros,
        })
    return in_maps


def _run(inputs, trace=False, reps=1, **trace_kw):
    import time
    from concourse.bass_utils import run_bass_kernel_spmd
    with_bv = bool(np.any(np.asarray(inputs["bv"])))
    nc = _get_nc(G, reps, with_bv)
    in_maps = _prep_inputs(**inputs)
    # The axon-tunneled devices occasionally fault transiently
    # (NRT_EXEC_UNIT_UNRECOVERABLE); the pool recovers on a fresh attempt.
    last_exc = None
    for attempt in range(4):
        try:
            res = run_bass_kernel_spmd(nc, in_maps,
                                       core_ids=list(range(NCORES)),
                                       trace=trace, **trace_kw)
            break
        except Exception as e:  # noqa: BLE001
            last_exc = e
            if "UNAVAILABLE" not in str(e) and "unrecoverable" not in str(e):
                raise
            time.sleep(10 * (attempt + 1))
    else:
        raise last_exc
    bout = np.asarray(inputs["bout"], np.float32)
    out = np.zeros((B, S, D), np.float32)
    for core in range(NCORES):
        out[core // 2] += res.results[core]["out"].astype(np.float32)
    out += bout[None, None, :]
    return out, res


def kernel(**inputs) -> np.ndarray:
    inputs = {k: np.asarray(v) for k, v in inputs.items()}
    out, _ = _run(inputs)
    return out


# revision 37
# speedup vs baseline: 21.0115x; 1.2391x over previous
# Trainium2 Bass kernel for MixedChunkAttention.
#
# Sharding: 8 cores = 4 batches x 2-way tensor-parallel split of INNER
# (E=2048 -> 1024 per core). Each core processes one full batch (the
# cross-chunk kv cumsum stays core-local) and one half of the inner dim;
# the host sums the two partial outputs per batch and adds bout.
#
# Precision scheme (rel_l2 ~= 1e-2 vs f32 reference, gate is 2e-2):
#   x@Wv runs 1-slot fp8e4m3 with the DoubleRow perf mode (2 K-tiles per
#   instruction -> 2x the fp32r matmul rate; measured on hw, the cost
#   model's 4x is wrong). Its quantization error washes out through the
#   attention averaging. The attn@v (quad) and k^T v (kv) matmuls are
#   also fp8 DoubleRow (attn weights, v, lin_k stored fp8).
#   Everything precision-critical (x@Wg, x@Win, attn QK^T, the v_lin
#   state matmul, o@Wout) stays fp32r.
#   Static scales: x_hi*8, Wv*256, lq*4, lk*4, attn*16, Wout/16 (so the
#   out psum holds the final value exactly); o stored as o*16 in f32r.
#
# Per-core dataflow (chunked over G=16 chunks of C=256 positions):
#   xT f32r + x_hi fp8 chunk-pair streamed to SBUF (host pre-quantizes)
#   xhT  = silu(Win^T @ xT)  f32r            [H, 2C]   (PE + ACT)
#   qqT/qkT/lqT/lkT f32r = per-partition affines       (DVE)
#   lk_nat = fp8(transpose(lkT))            [C, H]    (PE f32r transpose)
#   attnT[m,n] = fp8(relu(qkT^T @ qqT)^2 * 16)        (PE f32r, ACT, DVE)
#   v    = fp8(silu(x_hi @ Wv8))  natural [C, E']     (PE fp8 DR + ACT)
#   gT   = silu(Wg^T @ xT)   f32r            [E', 2C]  (PE + ACT)
#   vqlT = St^T @ lqT (f32r) + v^T @ attnT (fp8 DR, psum-fused)
#   oT   = (gT * vqlT) f32r (= o*16)                  (DVE)
#   St  += lk_nat^T @ v  (fp8 DR; St stores kv*4 in f32r)
#   out[c,:] = oT^T @ (Wout/16)  f32r -> bf16 stage -> DMA
#
# The output projection of chunk g is emitted during chunk g+1 so the
# in-order PE queue always has independent work while the oT chain
# (DVE/ACT) drains; kv is emitted before the quad matmuls so the St
# update lands ahead of the oT ops in the DVE queue.

import numpy as np

B, S, D = 4, 4096, 1024
C, H, E = 256, 128, 2048
G = S // C            # 16 chunks
ELOC = E // 2         # per-core inner slice
T = D // 128          # 8 d-tiles
ET = ELOC // 128      # 8 e-tiles
NCORES = 8

SX = 8.0              # x_hi stored scale
SW = 256.0            # Wv stored scale
SA = 16.0             # attn stored scale (sqrt folded into mask values)
SLK = 4.0             # lin_k stored scale
SLQ = 4.0             # lin_q stored scale
# oT holds o*16 (= vql*16 * gate); Wout shipped as Wout/16.

_CACHE = {}


def _build_nc(n_chunks=G, reps=1, with_bv=True):
    import concourse.mybir as mybir
    import concourse.tile as tile
    from concourse import bacc

    F32, F32R, BF16 = mybir.dt.float32, mybir.dt.float32r, mybir.dt.bfloat16
    FP8 = mybir.dt.float8e4
    AF = mybir.ActivationFunctionType
    OP = mybir.AluOpType
    DR = mybir.MatmulPerfMode.DoubleRow

    nc = bacc.Bacc()
    xt_d = nc.declare_dram_parameter("xT", [128, T, S], F32R, isOutput=False)
    xh_d = nc.declare_dram_parameter("xh8", [128, T, S], FP8, isOutput=False)
    wvh_d = nc.declare_dram_parameter("wvh", [128, T, ELOC], FP8, isOutput=False)
    wg_d = nc.declare_dram_parameter("wg", [128, T, ELOC], F32R, isOutput=False)
    win_d = nc.declare_dram_parameter("win", [128, T, H], F32R, isOutput=False)
    wout_d = nc.declare_dram_parameter("wout", [128, ET, D], F32R,
                                       isOutput=False)
    bv_d = nc.declare_dram_parameter("bv", [1, ELOC], F32R, isOutput=False)
    one_d = nc.declare_dram_parameter("ones", [1, 128], F32R, isOutput=False)
    bgt_d = nc.declare_dram_parameter("bgt", [128, ET], F32, isOutput=False)
    aff_d = nc.declare_dram_parameter("aff", [128, 9], F32, isOutput=False)
    msk_d = nc.declare_dram_parameter("masks", [128, 2, C], F32, isOutput=False)
    id_d = nc.declare_dram_parameter("ident", [128, 128], F32R, isOutput=False)
    zs_d = nc.declare_dram_parameter("zeros", [128, ELOC], F32R, isOutput=False)
    out_d = nc.declare_dram_parameter("out", [S, D], BF16, isOutput=True)

    with tile.TileContext(nc) as tc:
        with tc.tile_pool(name="wpool", bufs=1) as wpool, \
             tc.tile_pool(name="spool", bufs=1) as spool, \
             tc.tile_pool(name="xtp", bufs=2) as xtp, \
             tc.tile_pool(name="vp", bufs=2) as vp, \
             tc.tile_pool(name="gp", bufs=1) as gp, \
             tc.tile_pool(name="otp", bufs=2) as otp, \
             tc.tile_pool(name="osp", bufs=2) as osp, \
             tc.tile_pool(name="smallp", bufs=2) as smallp, \
             tc.tile_pool(name="ps512", bufs=4, space="PSUM") as ps512, \
             tc.tile_pool(name="pso", bufs=2, space="PSUM") as pso, \
             tc.tile_pool(name="ps256", bufs=1, space="PSUM") as ps256:

            # ---- persistent tiles ----
            wvh_sb = wpool.tile([128, T, ELOC], FP8, name="wvh_sb")
            wg_sb = wpool.tile([128, T, ELOC], F32R, name="wg_sb")
            win_sb = wpool.tile([128, T, H], F32R, name="win_sb")
            wout_sb = wpool.tile([128, ET, D], F32R, name="wout_sb")
            bv_sb = wpool.tile([1, ELOC], F32R, name="bv_sb")
            one_sb = wpool.tile([1, 128], F32R, name="one_sb")
            bgt_sb = wpool.tile([128, ET], F32, name="bgt_sb")
            aff_sb = wpool.tile([128, 9], F32, name="aff_sb")
            msk_sb = wpool.tile([128, 2, C], F32, name="msk_sb")
            ident = wpool.tile([128, 128], F32R, name="ident")
            for sb, d in ((win_sb, win_d), (wg_sb, wg_d)):
                nc.sync.dma_start(out=sb[:], in_=d[:])
            for sb, d in ((aff_sb, aff_d), (wvh_sb, wvh_d), (wout_sb, wout_d)):
                nc.scalar.dma_start(out=sb[:], in_=d[:])
            for sb, d in ((msk_sb, msk_d), (ident, id_d), (bgt_sb, bgt_d),
                          (bv_sb, bv_d), (one_sb, one_d)):
                nc.gpsimd.dma_start(out=sb[:], in_=d[:])

            St = spool.tile([128, ELOC], F32R, name="St")

            import contextlib
            rep_ctx = tc.For_i(0, reps) if reps > 1 else contextlib.nullcontext()
            with rep_ctx:
                nc.sync.dma_start(out=St[:], in_=zs_d[:])
                _chunk_body(nc, tc, n_chunks, with_bv, locals())

    nc.finalize()
    return nc


def _chunk_body(nc, tc, n_chunks, with_bv, env):
    import concourse.mybir as mybir
    F32, F32R, BF16 = mybir.dt.float32, mybir.dt.float32r, mybir.dt.bfloat16
    FP8 = mybir.dt.float8e4
    AF = mybir.ActivationFunctionType
    OP = mybir.AluOpType
    DR = mybir.MatmulPerfMode.DoubleRow
    (xt_d, xh_d, out_d) = (env["xt_d"], env["xh_d"], env["out_d"])
    (wvh_sb, wg_sb, win_sb, wout_sb, bv_sb, one_sb, bgt_sb, aff_sb,
     msk_sb, ident, St) = (
        env[k] for k in
        ["wvh_sb", "wg_sb", "win_sb", "wout_sb", "bv_sb", "one_sb",
         "bgt_sb", "aff_sb", "msk_sb", "ident", "St"])
    (xtp, vp, gp, otp, osp, smallp, ps512, pso, ps256) = (
        env[k] for k in ["xtp", "vp", "gp", "otp", "osp", "smallp",
                         "ps512", "pso", "ps256"])
    T2 = T // 2
    ZIN = 1.0 / (SX * SW)     # v psum holds z * SX*SW
    assert n_chunks % 2 == 0

    def emit_out(g, oT):
        # ---- output projection f32r (chunk g); psum = out exactly ----
        ostage = osp.tile([128, 2, D], BF16, name="ostage", tag="ostage")
        for ci in range(2):
            cb = slice(ci * 128, (ci + 1) * 128)
            for d2 in range(2):
                ds = slice(d2 * 512, (d2 + 1) * 512)
                o_ps = pso.tile([128, 512], F32, name="o_ps", tag="pso")
                for et in range(ET):
                    nc.tensor.matmul(o_ps[:], oT[:, et, cb],
                                     wout_sb[:, et, ds],
                                     start=(et == 0), stop=(et == ET - 1))
                nc.scalar.activation(ostage[:, ci, ds], o_ps[:], AF.Copy,
                                     bias=0.0)
            nc.sync.dma_start(
                out=out_d[g * C + ci * 128: g * C + (ci + 1) * 128, :],
                in_=ostage[:, ci, :])

    pending = None            # (g, oT) awaiting output projection
    for gp_i in range(n_chunks // 2):
            # ---- load x chunk-pair: f32r transposed + fp8 hi ----
            cs = slice(gp_i * 2 * C, (gp_i + 1) * 2 * C)
            xt = xtp.tile([128, T, 2 * C], F32R, name="xt", tag="xt")
            xht = xtp.tile([128, T, 2 * C], FP8, name="xht", tag="xht")
            nc.sync.dma_start(out=xt[:], in_=xt_d[:, :, cs])
            nc.sync.dma_start(out=xht[:], in_=xh_d[:, :, cs])

            # ---- xh projection f32r ----
            xh2 = smallp.tile([128, 2 * C], F32, name="xh2", tag="xh2", bufs=1)
            xh_ps = ps512.tile([128, 2 * C], F32, name="xh_ps", tag="ps512")
            for t in range(T):
                nc.tensor.matmul(xh_ps[:], win_sb[:, t, :], xt[:, t, :],
                                 start=(t == 0), stop=(t == T - 1))
            nc.scalar.activation(xh2[:], xh_ps[:], AF.Silu,
                                 bias=aff_sb[:, 8:9])

            # ---- gate projection f32r ----
            gT2 = gp.tile([128, ET, 2 * C], F32, name="gT2", tag="gT2")
            for et in range(ET):
                es = slice(et * 128, (et + 1) * 128)
                g_ps = ps512.tile([128, 2 * C], F32, name="g_ps", tag="ps512")
                for t in range(T):
                    nc.tensor.matmul(g_ps[:], wg_sb[:, t, es], xt[:, t, :],
                                     start=(t == 0), stop=(t == T - 1))
                nc.scalar.activation(gT2[:, et, :], g_ps[:], AF.Silu,
                                     bias=bgt_sb[:, et:et + 1])

            for gi in range(2):
                g = gp_i * 2 + gi
                co = gi * C                      # column offset into pair slabs
                xh = xh2[:, co:co + C]

                # ---- v natural [C, ELOC], 1-slot fp8 DoubleRow ----
                v_sb = vp.tile([128, 2, ELOC], FP8, name="v_sb", tag="v_sb")
                for ci in range(2):
                    cb = slice(co + ci * 128, co + (ci + 1) * 128)
                    for e2 in range(2):
                        v_ps = ps512.tile([128, 512], F32, name="v_ps",
                                          tag="ps512")
                        if with_bv:
                            nc.tensor.matmul(v_ps[:], one_sb[0:1, :],
                                             bv_sb[0:1, e2 * 512:(e2 + 1) * 512],
                                             start=True, stop=False)
                        for kp in range(T2):
                            ks = slice(2 * kp, 2 * kp + 2)
                            nc.tensor.matmul(
                                v_ps[:], xht[:, ks, cb],
                                wvh_sb[:, ks, e2 * 512:(e2 + 1) * 512],
                                start=(kp == 0 and not with_bv),
                                stop=(kp == T2 - 1), perf_mode=DR)
                        nc.scalar.activation(
                            v_sb[:, ci, e2 * 512:(e2 + 1) * 512], v_ps[:],
                            AF.Silu, bias=0.0, scale=ZIN)

                # ---- affines (lq*4 and lk*4 folded into aff) ----
                qqT = smallp.tile([128, C], F32R, name="qqT", tag="qqT")
                qkT = smallp.tile([128, C], F32R, name="qkT", tag="qkT")
                lqT = smallp.tile([128, C], F32R, name="lqT", tag="lqT")
                lkT = smallp.tile([128, C], F32R, name="lkT", tag="lkT", bufs=1)
                nc.vector.tensor_scalar(out=qqT[:], in0=xh[:],
                                        scalar1=aff_sb[:, 0:1], scalar2=aff_sb[:, 1:2],
                                        op0=OP.mult, op1=OP.add)
                nc.vector.tensor_scalar(out=qkT[:], in0=xh[:],
                                        scalar1=aff_sb[:, 2:3], scalar2=aff_sb[:, 3:4],
                                        op0=OP.mult, op1=OP.add)
                nc.vector.tensor_scalar(out=lqT[:], in0=xh[:],
                                        scalar1=aff_sb[:, 4:5], scalar2=aff_sb[:, 5:6],
                                        op0=OP.mult, op1=OP.add)
                nc.vector.tensor_scalar(out=lkT[:], in0=xh[:],
                                        scalar1=aff_sb[:, 6:7], scalar2=aff_sb[:, 7:8],
                                        op0=OP.mult, op1=OP.add)

                # ---- lk natural via PE transpose (f32r), cvt to fp8 ----
                lkn = smallp.tile([128, 2, H], FP8, name="lkn", tag="lkn")
                tr_ps = ps256.tile([128, 2, 128], F32R, name="tr_ps",
                                   tag="trps")
                for ci in range(2):
                    nc.tensor.matmul(tr_ps[:, ci, :],
                                     lkT[:, ci * 128:(ci + 1) * 128],
                                     ident[:], is_transpose=True,
                                     start=(ci == 0), stop=(ci == 1))
                nc.vector.tensor_copy(lkn[:, 0:2, :], tr_ps[:, 0:2, :])

                # ---- chunk attention attnT[m, n], stored attn*16 ----
                attnT = smallp.tile([128, 2, C], FP8, name="attnT", tag="attnT")
                at_ps = ps256.tile([128, 2, C], F32, name="at_ps", tag="atps")
                for mi in range(2):
                    nc.tensor.matmul(at_ps[:, mi, :],
                                     qkT[:, mi * 128:(mi + 1) * 128],
                                     qqT[:], start=(mi == 0), stop=(mi == 1))
                rt = smallp.tile([128, 2, C], F32, name="rt", tag="rt")
                nc.scalar.activation(rt[:], at_ps[:, 0:2, :], AF.Relu,
                                     bias=0.0)
                nc.vector.tensor_tensor(out=rt[:], in0=rt[:],
                                        in1=msk_sb[:, 0:2, :], op=OP.mult)
                nc.vector.tensor_tensor(out=attnT[:, 0:2, :], in0=rt[:],
                                        in1=rt[:], op=OP.mult)

                # ---- vql = lin (f32r) + quad (fp8 DR); oT = gate * vql ----
                # All lin matmuls first so the PE has cover while the
                # relu/mask/square chain produces attnT.
                oT = otp.tile([128, ET, C], F32R, name="oT", tag="oT")
                vql_pss = []
                for ep in range(ET // 2):
                    vql_ps = ps512.tile([128, 2, C], F32, name="vql_ps",
                                        tag="ps512")
                    vql_pss.append(vql_ps)
                    for j in range(2):
                        et = 2 * ep + j
                        es = slice(et * 128, (et + 1) * 128)
                        nc.tensor.matmul(vql_ps[:, j, :], St[:, es], lqT[:],
                                         start=(j == 0), stop=False)

                # ---- kv state update St += lk_nat^T @ v (fp8 DR) ----
                # Before the quad matmuls so the St add lands ahead of the
                # oT ops in the DVE queue (next chunk's lin matmuls wait).
                for e2 in range(2):
                    kv_ps = ps512.tile([128, 512], F32, name="kv_ps",
                                       tag="ps512")
                    nc.tensor.matmul(kv_ps[:], lkn[:, 0:2, :],
                                     v_sb[:, 0:2, e2 * 512:(e2 + 1) * 512],
                                     start=True, stop=True, perf_mode=DR)
                    nc.vector.tensor_tensor(out=St[:, e2 * 512:(e2 + 1) * 512],
                                            in0=St[:, e2 * 512:(e2 + 1) * 512],
                                            in1=kv_ps[:], op=OP.add)

                for ep in range(ET // 2):
                    vql_ps = vql_pss[ep]
                    for j in range(2):
                        et = 2 * ep + j
                        es = slice(et * 128, (et + 1) * 128)
                        nc.tensor.matmul(vql_ps[:, j, :], v_sb[:, 0:2, es],
                                         attnT[:, 0:2, :],
                                         start=False, stop=(j == 1),
                                         perf_mode=DR)
                    ee = slice(2 * ep, 2 * ep + 2)
                    nc.vector.tensor_tensor(
                        out=oT[:, ee, :], in0=gT2[:, ee, co:co + C],
                        in1=vql_ps[:, 0:2, :], op=OP.mult)

                # ---- previous chunk's output projection (pipelined) ----
                if pending is not None:
                    emit_out(*pending)
                pending = (g, oT)

    if pending is not None:
        emit_out(*pending)


def _get_nc(n_chunks=G, reps=1, with_bv=True):
    key = ("nc", n_chunks, reps, with_bv)
    if key not in _CACHE:
        _CACHE[key] = _build_nc(n_chunks, reps, with_bv)
    return _CACHE[key]


def _fp8(a):
    import ml_dtypes
    return np.asarray(a, np.float32).astype(ml_dtypes.float8_e4m3)


def _prep_inputs(x, Wv, bv, Wg, bg, Win, bin_, Wout, bout,
                 g_qq, b_qq, g_qk, b_qk, g_lq, b_lq, g_lk, b_lk):
    f = np.float32
    scale = f(E) ** f(0.5)
    tri = np.triu(np.ones((128, 128), f))          # keep p <= col
    sa = f(SA) ** f(0.5)                           # folded into mask values
    masks = np.zeros((128, 512), f)
    masks[:, 0:128] = tri * sa
    masks[:, 128:256] = sa
    masks[:, 256:384] = 0.0
    masks[:, 384:512] = tri * sa
    aff = np.stack([
        g_qq / scale, b_qq / scale, g_qk, b_qk,
        g_lq * SLQ, b_lq * SLQ, g_lk * SLK, b_lk * SLK, bin_],
        axis=1).astype(f)                          # [128, 9]
    ones = np.ones((1, 128), f)
    zeros = np.zeros((128, ELOC), f)

    def dtile(w, n):          # [D, n] -> [128, T, n]
        return np.ascontiguousarray(w.reshape(T, 128, n).transpose(1, 0, 2))

    x = np.asarray(x, f)
    WvH = _fp8(np.asarray(Wv, f) * SW)
    Wg = np.asarray(Wg, f)
    Win = np.asarray(Win, f)
    # oT holds o*16, so ship Wout/16 and the out psum is exact.
    Wout16 = np.asarray(Wout, f) / 16.0

    in_maps = []
    for core in range(NCORES):
        b, h = core // 2, core % 2
        sl = slice(h * ELOC, (h + 1) * ELOC)
        xb = x[b]                                      # [S, D]
        xh8 = _fp8(xb * SX)
        def xtile(a):          # [S, D] -> [128, T, S]
            return np.ascontiguousarray(
                a.T.reshape(T, 128, S).transpose(1, 0, 2))
        wout_l = np.ascontiguousarray(
            Wout16[sl, :].reshape(ET, 128, D).transpose(1, 0, 2))
        in_maps.append({
            "xT": xtile(xb).astype(f),
            "xh8": xtile(xh8),
            "wvh": dtile(WvH[:, sl], ELOC),
            "wg": dtile(Wg[:, sl], ELOC).astype(f),
            "win": dtile(Win, H).astype(f),
            "wout": wout_l.astype(f),
            "bv": (np.asarray(bv, f) * (SX * SW))[sl].reshape(1, ELOC),
            "ones": ones,
            "bgt": np.ascontiguousarray(
                np.asarray(bg, f)[sl].reshape(ET, 128).T),
            "aff": aff,
            "masks": masks.reshape(128, 2, C),
            "ident": np.eye(128, dtype=f),
            "zeros": zeros,
        })
    return in_maps


def _run(inputs, trace=False, reps=1, **trace_kw):
    import time
    from concourse.bass_utils import run_bass_kernel_spmd
    with_bv = bool(np.any(np.asarray(inputs["bv"])))
    nc = _get_nc(G, reps, with_bv)
    in_maps = _prep_inputs(**inputs)
    # The axon-tunneled devices occasionally fault transiently
    # (NRT_EXEC_UNIT_UNRECOVERABLE); the pool recovers on a fresh attempt.
    last_exc = None
    for attempt in range(4):
        try:
            res = run_bass_kernel_spmd(nc, in_maps,
                                       core_ids=list(range(NCORES)),
                                       trace=trace, **trace_kw)
            break
        except Exception as e:  # noqa: BLE001
            last_exc = e
            if "UNAVAILABLE" not in str(e) and "unrecoverable" not in str(e):
                raise
            time.sleep(10 * (attempt + 1))
    else:
        raise last_exc
    bout = np.asarray(inputs["bout"], np.float32)
    out = np.zeros((B, S, D), np.float32)
    for core in range(NCORES):
        out[core // 2] += res.results[core]["out"].astype(np.float32)
    out += bout[None, None, :]
    return out, res


def kernel(**inputs) -> np.ndarray:
    inputs = {k: np.asarray(v) for k, v in inputs.items()}
    out, _ = _run(inputs)
    return out
